# revision 15
# baseline (speedup 1.0000x reference)
"""Trainium2 Bass kernel for nn_CoucheinitialeGNN (GNN edge-MLP + segment-sum normalize).

Math (reference):
    bucket = clip(int(dist), 0, 9); one_hot [E,10]
    h      = relu(dist @ W1 + b1)          [E,128]
    mlp    = relu(h @ W2 + b2)             [E,54]
    w      = concat([one_hot, mlp])        [E,64]
    d      = segment_sum(w, src, N)        [N,64]
    out    = w / d[src]   (0/0 := 0)       [E,64]

Strategy (feature-major, fp16-limb transposed L2): shard nodes across 8 cores;
per core sort nodes by degree and pack 128 nodes per bin (column = (slot j,
node p), j-outer).  L1 computes h for the KINKY relu features exactly in f32
PSUM via a 10-row bf16 limb matmul; hinge features u_c = relu(bucket - c) ride
along so the 10-bucket one-hot comes out of L2 as exact tent combinations
(u_{k-1} - 2u_k + u_{k+1}).  relu(p1) is split into an fp16 hi limb (Scalar /
Vector relu pass) and an fp16 residual limb (Vector max-sub pass); L2 is THREE
fp16 matmul streams per chunk with stationary [*, 64] weights -- hh@W2hi,
hh@W2lo, hl@W2hi -- accumulated in one PSUM tile (combined pre-relu accuracy
~2^-21, fp16 matmuls run 1 cyc/row vs fp32's ~8).  Linear relu features fold
exactly into dist/const fp16-limb rows.  Bins with equal padded degree are
processed in PAIRS via column tiling (tile_position (0,0)/(0,64)), so the
relu/segment-tree/normalize stages run at full 128-partition width.  The
segment-sum halving tree runs on GpSimd (otherwise idle), the final
eps-add/reciprocal/normalize on Vector.  Outputs are bf16 [64, EP]
feature-major; the host transposes + scatters rows back.
"""

import numpy as np
import ml_dtypes

import concourse.bass as bass
import concourse.bacc as bacc
import concourse.tile as tile
import concourse.mybir as mybir
from concourse.bass_utils import run_bass_kernel_spmd

F32 = mybir.dt.float32
BF16 = mybir.dt.bfloat16
FP16 = mybir.dt.float16

N_NODES = 100000
N_EDGES = 1600000
N_CORES = 8
THRESHOLD = 10.0

NPC = N_NODES // N_CORES                     # 12500 nodes per core
NBIN = -(-NPC // 128)                        # 98 bins of 128 nodes
EPS = 2e-38                                  # keeps 1/(d+eps) finite + normal
CHUNK = 512                                  # matmul N (one PSUM bank)
QUAD = 1024                                  # p1 PSUM tile width (2 banks)
L2W = 1024                                   # psum2 tile width (2 banks)

# engine-split tuning: of every N relu chunks, first K go to Scalar
RELU1_ACT = (8, 8)
RELU2_ACT = (3, 4)
MULT_POOL = (1, 2)                           # fraction of pair normalizes on gpsimd
TREE_POOL = (1, 1)                           # fraction of pair trees on gpsimd


# ---------------------------------------------------------------------------
# host-side weight folding
# ---------------------------------------------------------------------------

def fold_weights(W1, b1, W2, b2):
    """Kinky relu features -> device; linear fold exactly into dist/const
    fp16-limb rows; dead drop.  Returns L1 bf16 [10, P1], three fp16 L2
    stationaries (A: hi, B: lo, C3: hi for the hl stream), and KH."""
    W1 = np.asarray(W1, np.float32).reshape(-1)       # [128]
    b1 = np.asarray(b1, np.float32).reshape(-1)       # [128]
    W2 = np.asarray(W2, np.float32)                   # [128, 54]
    b2 = np.asarray(b2, np.float32).reshape(-1)       # [54]

    lo = b1
    hi = THRESHOLD * W1 + b1
    with np.errstate(divide="ignore", invalid="ignore"):
        t = np.where(W1 != 0, -b1 / W1, np.inf)
    kinky = (t > -0.5) & (t < THRESHOLD + 0.5) & (W1 != 0)
    dead = ~kinky & (np.maximum(lo, hi) <= 0)
    linear = ~kinky & ~dead                            # relu == identity on (0,10]

    KH = int(kinky.sum())
    P1 = KH + 11                                       # kinky | u_0..u_9 | id(bk)
    P2 = P1 + 3                                        # + dh16, dm16, ones rows
    assert P2 <= 128, f"P2={P2} too large"

    A = (W2[linear].astype(np.float64) * W1[linear, None].astype(np.float64)).sum(0)
    C = (W2[linear].astype(np.float64) * b1[linear, None].astype(np.float64)).sum(0) \
        + b2.astype(np.float64)

    def split3(v):
        hi_ = v.astype(ml_dtypes.bfloat16)
        r = v - hi_.astype(np.float32)
        mid = r.astype(ml_dtypes.bfloat16)
        lo_ = (r - mid.astype(np.float32)).astype(ml_dtypes.bfloat16)
        return hi_, mid, lo_

    wh, wm, wl = split3(W1[kinky])
    bh, bm, bl = split3(b1[kinky])
    # L1 rhs rows: [dh, dh, dh, dm, dm, dl, v, v, v, bk]
    l1 = np.zeros((10, P1), ml_dtypes.bfloat16)
    for i, row in enumerate([wh, wm, wl, wh, wm, wh, bh, bm, bl]):
        l1[i, :KH] = row
    for c in range(10):                                # u_c = relu(bk - c)
        l1[6, KH + c] = -float(c)
        l1[9, KH + c] = 1.0
    l1[9, KH + 10] = 1.0                               # id col: p1 = bk

    def f16(v):
        return np.asarray(v, np.float64).astype(np.float16)

    W2k = W2[kinky]
    W2k_hi = f16(W2k)
    W2k_lo = f16(W2k - W2k_hi.astype(np.float32))
    Ah = f16(A)
    Al = f16(A - Ah.astype(np.float64))
    Ch = f16(C)
    Cl = f16(C - Ch.astype(np.float64))

    HG = KH
    w2a = np.zeros((P2, 64), np.float16)               # hi stream over rhs2
    # one-hot tents (cols 0..9), exact small-int coefficients
    w2a[HG + 10, 0] = 1.0                              # id (bk)
    w2a[P1 + 2, 0] = 1.0                               # ones row
    w2a[HG + 0, 0] = -2.0
    w2a[HG + 1, 0] = 1.0
    for k in range(1, 9):
        w2a[HG + k - 1, k] = 1.0
        w2a[HG + k, k] = -2.0
        w2a[HG + k + 1, k] = 1.0
    w2a[HG + 8, 9] = 1.0                               # u_10 == 0 always
    w2a[HG + 9, 9] = -2.0
    # MLP cols 10..63
    w2a[:KH, 10:64] = W2k_hi
    w2a[P1 + 0, 10:64] = Ah                            # dh16 row
    w2a[P1 + 1, 10:64] = Ah                            # dm16 row
    w2a[P1 + 2, 10:64] = Ch                            # ones row

    w2b = np.zeros((P2, 64), np.float16)               # lo stream over rhs2
    w2b[:KH, 10:64] = W2k_lo
    w2b[P1 + 0, 10:64] = Al
    w2b[P1 + 1, 10:64] = Al
    w2b[P1 + 2, 10:64] = Cl

    w2c3 = np.zeros((KH, 64), np.float16)              # hi stream over hl
    w2c3[:, 10:64] = W2k_hi
    return l1, w2a, w2b, w2c3, KH


# ---------------------------------------------------------------------------
# host-side edge partitioning (node-major bins)
# ---------------------------------------------------------------------------

def plan(src):
    """Sort edges by src, shard nodes across cores, sort nodes by degree and
    pack 128 per bin.  Returns per-core edge->slot data and the shared
    per-bin padded degree profile D (even, identical across cores)."""
    order = np.argsort(src, kind="stable")
    ssrc = src[order]
    bounds = np.searchsorted(ssrc, np.arange(N_CORES + 1) * NPC)
    cores = []
    Dmat = np.zeros((N_CORES, NBIN), np.int64)
    for k in range(N_CORES):
        lo, hi = bounds[k], bounds[k + 1]
        eids = order[lo:hi]
        lsrc = (ssrc[lo:hi] - k * NPC).astype(np.int64)
        deg = np.bincount(lsrc, minlength=NPC)
        nodeord = np.argsort(-deg, kind="stable")
        rank = np.empty(NPC, np.int64)
        rank[nodeord] = np.arange(NPC)
        degs = deg[nodeord]
        dpad = np.zeros(NBIN * 128, np.int64)
        dpad[:NPC] = degs
        Dmat[k] = dpad.reshape(NBIN, 128).max(1)
        starts = np.concatenate([[0], np.cumsum(deg)])
        j = np.arange(len(lsrc)) - starts[lsrc]
        cores.append({"eids": eids, "lsrc": lsrc, "rank": rank, "j": j})
    Dm = Dmat.max(0)
    D = Dm + (Dm & 1)                        # even so halving trees stay simple
    cbase = np.concatenate([[0], np.cumsum(128 * D)])
    dbase = np.concatenate([[0], np.cumsum(D)])
    return cores, D, cbase, dbase, int(cbase[-1]), int(dbase[-1])


def prepare(cores, D, cbase, dbase, EP, DSUM, dist):
    in_maps = []
    gids_all = []
    for c in cores:
        eids, lsrc, j = c["eids"], c["lsrc"], c["j"]
        r = c["rank"][lsrc]
        p = r % 128
        b = r // 128
        col = cbase[b] + j * 128 + p             # device column of this edge

        de = dist[eids]
        distv = np.zeros(EP, np.float32)
        distv[col] = de
        valid = np.zeros(EP, np.float32)
        valid[col] = 1.0
        dh = distv.astype(ml_dtypes.bfloat16)
        r1 = distv - dh.astype(np.float32)
        dm = r1.astype(ml_dtypes.bfloat16)
        dl = (r1 - dm.astype(np.float32)).astype(ml_dtypes.bfloat16)
        bk = np.zeros(EP, np.float32)
        bk[col] = np.clip(de.astype(np.int32), 0, 9)

        rl1 = np.empty((10, EP), ml_dtypes.bfloat16)
        rl1[0] = dh
        rl1[1] = dh
        rl1[2] = dh
        rl1[3] = dm
        rl1[4] = dm
        rl1[5] = dl
        rl1[6] = valid
        rl1[7] = valid
        rl1[8] = valid
        rl1[9] = bk.astype(ml_dtypes.bfloat16)

        dh16 = distv.astype(np.float16)
        dm16 = (distv - dh16.astype(np.float32)).astype(np.float16)
        ex = np.empty((3, EP), np.float16)
        ex[0] = dh16
        ex[1] = dm16
        ex[2] = valid.astype(np.float16)

        gids = np.full(EP, -1, np.int64)
        gids[col] = eids
        in_maps.append({"rl1": rl1, "ex": ex})
        gids_all.append(gids)
    return in_maps, gids_all


def make_pairs(D):
    """Pair adjacent equal-degree bins (D is sorted desc, equal runs adjacent)."""
    pairs = []
    b = 0
    while b < NBIN:
        if D[b] == 0:
            b += 1
            continue
        if b + 1 < NBIN and D[b + 1] == D[b]:
            pairs.append((b, 2))
            b += 2
        else:
            pairs.append((b, 1))
            b += 1
    return pairs


# ---------------------------------------------------------------------------
# device kernel
# ---------------------------------------------------------------------------

_NC_CACHE = {}


def build_kernel(KH, l1_np, w2a_np, w2b_np, w2c3_np, D, cbase, EP):
    P1 = KH + 11
    P2 = P1 + 3
    nc = bacc.Bacc("TRN2", target_bir_lowering=False, debug=False, num_devices=N_CORES)

    rl1_d = nc.dram_tensor("rl1", [10, EP], BF16, kind="ExternalInput")
    ex_d = nc.dram_tensor("ex", [3, EP], FP16, kind="ExternalInput")
    out_d = nc.dram_tensor("out", [64, EP], BF16, kind="ExternalOutput")

    l1_stack = np.zeros((42, P1), ml_dtypes.bfloat16)
    for _s in range(2):
        l1_stack[32 * _s:32 * _s + 10] = l1_np
    l1_t = nc.inline_tensor(np.ascontiguousarray(l1_stack), name="l1w")
    w2a_t = nc.inline_tensor(np.ascontiguousarray(w2a_np), name="w2a")
    w2b_t = nc.inline_tensor(np.ascontiguousarray(w2b_np), name="w2b")
    w2c3_t = nc.inline_tensor(np.ascontiguousarray(w2c3_np), name="w2c3")

    Relu = mybir.ActivationFunctionType.Relu
    ADD = mybir.AluOpType.add
    MULT = mybir.AluOpType.mult
    MAX = mybir.AluOpType.max
    SUB = mybir.AluOpType.subtract
    pairs = make_pairs(D)

    with tile.TileContext(nc) as tc, nc.allow_low_precision(
        reason="fp16/bf16 limb sums and bf16 outputs within the 2e-2 tolerance"
    ):
        with (
            tc.tile_pool(name="const", bufs=1) as cpool,
            tc.tile_pool(name="rio", bufs=2) as riop,
            tc.tile_pool(name="rhs2", bufs=2) as r2p,
            tc.tile_pool(name="hlp", bufs=2) as hlp,
            tc.tile_pool(name="wk", bufs=2) as wkp,
            tc.tile_pool(name="sm", bufs=2) as smp,
            tc.tile_pool(name="ps1", bufs=2, space="PSUM") as ps1p,
            tc.tile_pool(name="ps2", bufs=2, space="PSUM") as ps2p,
        ):
            l1c = cpool.tile([42, P1], BF16)
            w2a = cpool.tile([P2, 64], FP16)
            w2b = cpool.tile([P2, 64], FP16)
            w2c3 = cpool.tile([KH, 64], FP16)
            nc.sync.dma_start(l1c[:], l1_t[:, :])
            nc.sync.dma_start(w2a[:], w2a_t[:, :])
            nc.sync.dma_start(w2b[:], w2b_t[:, :])
            nc.sync.dma_start(w2c3[:], w2c3_t[:, :])

            k1 = [0]
            k2 = [0]

            def phase1(pi, b0, nb):
                """L1 matmuls + fp16 hi/lo split: fills rhs2 + hlt for a pair."""
                Db = int(D[b0])
                S = 128 * Db
                T = nb * S
                e0 = int(cbase[b0])

                rhs2 = r2p.tile([P2, T], FP16, tag="rhs2", name="rhs2")
                hlt = hlp.tile([KH, T], FP16, tag="hl", name="hlt")
                rl1 = riop.tile([42, T], BF16, tag="rl1", name="rl1")
                for sl in range(2):
                    nc.sync.dma_start(rl1[32 * sl:32 * sl + 10, :],
                                      rl1_d[:, e0:e0 + T])
                nc.sync.dma_start(rhs2[P1:P1 + 3, :], ex_d[:, e0:e0 + T])

                for q0 in range(0, T, QUAD):
                    qw = min(QUAD, T - q0)
                    p1 = ps1p.tile([P1, QUAD], F32, tag="p1", name="p1")
                    c0 = 0
                    sl = 0
                    while c0 < qw:
                        cw = min(CHUNK, qw - c0)
                        nc.tensor.matmul(
                            out=p1[:, c0:c0 + cw],
                            lhsT=l1c[32 * sl:32 * sl + 10, :],
                            rhs=rl1[32 * sl:32 * sl + 10, q0 + c0:q0 + c0 + cw],
                            start=True, stop=True,
                            tile_position=(32 * sl, 0),
                        )
                        c0 += cw
                        sl ^= 1
                    hdst = rhs2[0:P1, q0:q0 + qw]
                    if k1[0] % RELU1_ACT[1] < RELU1_ACT[0]:
                        nc.scalar.activation(hdst, p1[:, :qw], Relu)
                    else:
                        nc.vector.tensor_scalar_max(hdst, p1[:, :qw], 0.0)
                    k1[0] += 1
                    # hl = relu(p1) - hh (fp16 residual), kinky rows only
                    nc.vector.scalar_tensor_tensor(
                        out=hlt[:, q0:q0 + qw], in0=p1[0:KH, :qw], scalar=0.0,
                        in1=rhs2[0:KH, q0:q0 + qw], op0=MAX, op1=SUB,
                    )
                return rhs2, hlt

            def phase2(pi, b0, nb, rhs2, hlt):
                """L2 streams + relu2 + segment tree + normalize + store."""
                Db = int(D[b0])
                S = 128 * Db
                e0 = int(cbase[b0])
                NP = 64 * nb

                w_t = wkp.tile([128, Db, 128], BF16, tag="w", name="w_t")
                for ci in range(0, S, L2W):
                    cw = min(L2W, S - ci)
                    nj = cw // 128
                    j0 = ci // 128
                    pm = ps2p.tile([128, L2W // 128, 128], F32, tag="pm")
                    for h2 in range(0, cw, CHUNK):
                        h2w = min(CHUNK, cw - h2)
                        po = slice(h2 // 128, (h2 + h2w) // 128)
                        for hx in range(nb):
                            g0 = hx * S + ci + h2
                            tp = (0, 64 * hx)
                            ps = slice(64 * hx, 64 * hx + 64)
                            nc.tensor.matmul(
                                out=pm[ps, po, :], lhsT=w2a[:],
                                rhs=rhs2[:, g0:g0 + h2w],
                                start=True, stop=False, tile_position=tp,
                            )
                            nc.tensor.matmul(
                                out=pm[ps, po, :], lhsT=w2b[:],
                                rhs=rhs2[:, g0:g0 + h2w],
                                start=False, stop=False, tile_position=tp,
                            )
                            nc.tensor.matmul(
                                out=pm[ps, po, :], lhsT=w2c3[:],
                                rhs=hlt[:, g0:g0 + h2w],
                                start=False, stop=True, tile_position=tp,
                            )
                    dst = w_t[0:NP, j0:j0 + nj, :]
                    if k2[0] % RELU2_ACT[1] < RELU2_ACT[0]:
                        nc.scalar.activation(dst, pm[0:NP, 0:nj, :], Relu)
                    else:
                        nc.vector.tensor_scalar_max(dst, pm[0:NP, 0:nj, :], 0.0)
                    k2[0] += 1

                # ---- segment sum: halving tree over j (GpSimd / Vector)
                teng = nc.gpsimd if pi % TREE_POOL[1] < TREE_POOL[0] else nc.vector
                dsb = smp.tile([128, 128], F32, tag="dsb", name="dsb")

                def final_add(a_ap, b_ap):
                    nc.vector.scalar_tensor_tensor(
                        out=dsb[0:NP, :].unsqueeze(1), in0=a_ap, scalar=EPS,
                        in1=b_ap, op0=ADD, op1=ADD,
                    )

                if Db == 2:
                    final_add(w_t[0:NP, 0:1, :], w_t[0:NP, 1:2, :])
                else:
                    red = wkp.tile([128, Db // 2, 128], BF16, tag="red", name="red")
                    srcv = w_t
                    cur = Db
                    while cur > 2:
                        half = cur // 2
                        teng.tensor_tensor(
                            out=red[0:NP, 0:half, :],
                            in0=srcv[0:NP, 0:half, :],
                            in1=srcv[0:NP, half:2 * half, :],
                            op=ADD,
                        )
                        if cur & 1:
                            teng.tensor_tensor(
                                out=red[0:NP, 0:1, :],
                                in0=red[0:NP, 0:1, :],
                                in1=srcv[0:NP, 2 * half:cur, :],
                                op=ADD,
                            )
                        srcv = red
                        cur = half
                    if cur == 2:
                        final_add(srcv[0:NP, 0:1, :], srcv[0:NP, 1:2, :])
                    else:                     # odd-add path collapsed to 1
                        nc.vector.tensor_scalar_add(
                            dsb[0:NP, :].unsqueeze(1), srcv[0:NP, 0:1, :], EPS,
                        )

                # ---- r = 1/(d+eps), bf16 for the 2x multiply
                rbf = smp.tile([128, 128], F32, tag="rbf", name="rbf")
                nc.vector.reciprocal_approx_fast(out=rbf[0:NP, :], in_=dsb[0:NP, :])
                rb = smp.tile([128, 128], BF16, tag="rb", name="rb")
                nc.vector.tensor_scalar_max(rb[0:NP, :], rbf[0:NP, :], 0.0)

                # ---- out = w * r (broadcast over j), bf16 store
                meng = nc.gpsimd if pi % MULT_POOL[1] < MULT_POOL[0] else nc.vector
                out_t = wkp.tile([128, Db, 128], BF16, tag="ot", name="out_t")
                meng.tensor_tensor(
                    out=out_t[0:NP, :, :],
                    in0=w_t[0:NP, :, :],
                    in1=rb[0:NP, :].unsqueeze(1).to_broadcast([NP, Db, 128]),
                    op=MULT,
                )
                nc.sync.dma_start(out_d[:, e0:e0 + S], out_t[0:64, :, :])
                if nb == 2:
                    nc.sync.dma_start(out_d[:, e0 + S:e0 + 2 * S], out_t[64:128, :, :])

            # software pipeline: phase1(p) runs one pair ahead of phase2(p-1)
            # so the PE's L2 never waits on the Act/Vector hi/lo split.
            prev = None
            for pi, (b0, nb) in enumerate(pairs):
                r2, hl2 = phase1(pi, b0, nb)
                if prev is not None:
                    phase2(*prev)
                prev = (pi, b0, nb, r2, hl2)
            if prev is not None:
                phase2(*prev)
    nc.compile()
    return nc


# ---------------------------------------------------------------------------
# entry point
# ---------------------------------------------------------------------------

def kernel(x, edge_index, edge_attr, W1, b1, W2, b2):
    src = np.asarray(edge_index)[0].astype(np.int64)
    dist = np.asarray(edge_attr, np.float32)[:, 0]

    l1_np, w2a_np, w2b_np, w2c3_np, KH = fold_weights(W1, b1, W2, b2)
    cores, D, cbase, dbase, EP, DSUM = plan(src)

    key = (KH, D.tobytes(), l1_np.tobytes(), w2a_np.tobytes())
    nc = _NC_CACHE.get(key)
    if nc is None:
        nc = build_kernel(KH, l1_np, w2a_np, w2b_np, w2c3_np, D, cbase, EP)
        _NC_CACHE[key] = nc

    in_maps, gids_all = prepare(cores, D, cbase, dbase, EP, DSUM, dist)
    res = run_bass_kernel_spmd(nc, in_maps, core_ids=list(range(N_CORES)))

    final = np.empty((N_EDGES, 64), np.float32)
    for k in range(N_CORES):
        o = np.asarray(res.results[k]["out"]).astype(np.float32)   # [64, EP]
        gids = gids_all[k]
        m = gids >= 0
        final[gids[m]] = o[:, m].T
    return final
